# revision 10
# baseline (speedup 1.0000x reference)
"""MoE router (linear gate -> softmax -> top-8 indices) on 8 Trainium2 cores.

Strategy (data-parallel over tokens, W replicated):
  - Each core gets 2048 tokens. x is pre-transposed on the host so each core
    receives x^T [4096, 2048] — the PE needs the contraction dim (d_model) on
    partitions and fp32 has no DMA-transpose path, so transposing on-chip
    would double PE work.
  - Softmax is strictly monotonic, so top-k of softmax(logits) == top-k of
    logits; the softmax is skipped entirely.
  - The gate matmul runs in float32r (fp20: 1+8+11) which streams at 1
    cycle/row vs fp32's 4, using an exactly-compensated split:
        x = x_hi + x_lo,  W = w_hi + w_lo   (each half fp20-representable)
        logits = w_hi·x_hi + w_lo·x_hi + w_hi·x_lo   (3 passes, fp32 PSUM)
    The dropped w_lo·x_lo term is O(2^-24) relative — fp32-level accuracy
    (validated on HW: max err 1.5e-7 vs fp32's 1.2e-7). W is split on the
    host; x is split on-chip (ACT rounds to f32r, DVE subtracts) to keep HBM
    traffic at 4 bytes/element.
  - Top-8: PE-transpose the [64, 512] logit tiles to [128 tokens, 64
    experts], then DVE Max8 / MaxIndex produce the 8 largest values and
    indices per token (descending, ties -> lowest index, matching
    jax.lax.top_k). Indices are staged in SBUF and written with one DMA.
"""

import numpy as np

import concourse.bass as bass
import concourse.mybir as mybir
import concourse.tile as tile
from concourse import bacc
from concourse.bass_utils import run_bass_kernel_spmd
from concourse.masks import make_identity

N_CORES = 8
N_TOKENS = 16384
D_MODEL = 4096
N_EXPERTS = 64
TOP_K = 8

TPC = N_TOKENS // N_CORES      # tokens per core (2048)
GROUP = 512                    # tokens per matmul (max 4-byte moving dim)
N_GROUPS = TPC // GROUP        # 4
N_CHUNK = D_MODEL // 128       # 32 contraction chunks
PAIR = 2                       # chunks fetched per DMA (2 MB transfers)
N_PAIR = N_CHUNK // PAIR       # 16
N_BLK = TPC // 128             # 16 x 128-token output blocks

F32 = mybir.dt.float32
F32R = mybir.dt.float32r
U32 = mybir.dt.uint32

_CACHE: dict = {}


def _build_program():
    nc = bacc.Bacc(
        "TRN2", target_bir_lowering=False, debug=False, num_devices=N_CORES
    )
    xt_d = nc.dram_tensor("xt", [D_MODEL, TPC], F32, kind="ExternalInput")
    # W^T pre-packed+split on host: [128, 32*64], row p, col k*64+e = W_{hi/lo}[e, k*128+p]
    wh_d = nc.dram_tensor("wh", [128, N_CHUNK * N_EXPERTS], F32R, kind="ExternalInput")
    wl_d = nc.dram_tensor("wl", [128, N_CHUNK * N_EXPERTS], F32R, kind="ExternalInput")
    # idx laid out [128 partitions, 16 blocks, 8] — host unpermutes to [2048, 8]
    idx_d = nc.dram_tensor("idx", [128, N_BLK * TOP_K], U32, kind="ExternalOutput")

    with tile.TileContext(nc) as tc:
        with (
            tc.tile_pool(name="const", bufs=1) as const_pool,
            tc.tile_pool(name="xin", bufs=3) as x_pool,
            tc.tile_pool(name="hi", bufs=3) as hi_pool,
            tc.tile_pool(name="lo", bufs=3) as lo_pool,
            tc.tile_pool(name="lg_ps", bufs=1, space="PSUM") as lg_ps_pool,
            tc.tile_pool(name="lg_sb", bufs=2) as lg_pool,
            tc.tile_pool(name="lt_ps", bufs=2, space="PSUM") as lt_ps_pool,
            tc.tile_pool(name="small", bufs=2 * 4) as small_pool,
        ):
            ident = const_pool.tile([128, 128], F32)
            make_identity(nc, ident[:])
            wh_sb = const_pool.tile([128, N_CHUNK, N_EXPERTS], F32R)
            nc.sync.dma_start(
                wh_sb[:], wh_d.ap().rearrange("p (k e) -> p k e", k=N_CHUNK)
            )
            wl_sb = const_pool.tile([128, N_CHUNK, N_EXPERTS], F32R)
            nc.sync.dma_start(
                wl_sb[:], wl_d.ap().rearrange("p (k e) -> p k e", k=N_CHUNK)
            )
            idx_stage = const_pool.tile([128, N_BLK, TOP_K], U32)

            lg_ps = [
                lg_ps_pool.tile([N_EXPERTS, GROUP], F32, name=f"lg{g}", tag=f"lg{g}")
                for g in range(N_GROUPS)
            ]

            xt_view = xt_d.ap().rearrange("(k p) t -> p k t", p=128)
            for pr in range(N_PAIR):
                x_sb = x_pool.tile([128, PAIR, TPC], F32)
                nc.sync.dma_start(
                    x_sb[:], xt_view[:, PAIR * pr : PAIR * (pr + 1), :]
                )
                hi = hi_pool.tile([128, PAIR, TPC], F32R)
                nc.scalar.copy(hi[:], x_sb[:])
                lo = lo_pool.tile([128, PAIR, TPC], F32R)
                nc.vector.tensor_tensor(
                    lo[:], x_sb[:], hi[:].bitcast(F32), mybir.AluOpType.subtract
                )
                for c in range(PAIR):
                    k = PAIR * pr + c
                    for g in range(N_GROUPS):
                        sl = slice(g * GROUP, (g + 1) * GROUP)
                        nc.tensor.matmul(
                            lg_ps[g][:], wh_sb[:, k], hi[:, c, sl],
                            start=(k == 0), stop=False,
                        )
                    for g in range(N_GROUPS):
                        sl = slice(g * GROUP, (g + 1) * GROUP)
                        nc.tensor.matmul(
                            lg_ps[g][:], wh_sb[:, k], lo[:, c, sl],
                            start=False, stop=False,
                        )
                    for g in range(N_GROUPS):
                        sl = slice(g * GROUP, (g + 1) * GROUP)
                        nc.tensor.matmul(
                            lg_ps[g][:], wl_sb[:, k], hi[:, c, sl],
                            start=False, stop=(k == N_CHUNK - 1),
                        )

            for g in range(N_GROUPS):
                lg_sb = lg_pool.tile([N_EXPERTS, GROUP], F32)
                nc.vector.tensor_copy(lg_sb[:], lg_ps[g][:])
                for b in range(GROUP // 128):
                    lt_ps = lt_ps_pool.tile([128, N_EXPERTS], F32)
                    nc.tensor.transpose(
                        lt_ps[:],
                        lg_sb[:, b * 128 : (b + 1) * 128],
                        ident[:N_EXPERTS, :N_EXPERTS],
                    )
                    lt_sb = small_pool.tile([128, N_EXPERTS], F32, tag="lt")
                    nc.vector.tensor_copy(lt_sb[:], lt_ps[:])
                    vals = small_pool.tile([128, TOP_K], F32, tag="vals")
                    nc.vector.max(vals[:], lt_sb[:])
                    nc.vector.max_index(
                        idx_stage[:, g * (GROUP // 128) + b, :], vals[:], lt_sb[:]
                    )

            nc.sync.dma_start(
                idx_d.ap().rearrange("p (b k) -> p b k", b=N_BLK), idx_stage[:]
            )

    nc.compile()
    return nc


def _get_program():
    if "nc" not in _CACHE:
        _CACHE["nc"] = _build_program()
    return _CACHE["nc"]


def _round_f32r(a: np.ndarray) -> np.ndarray:
    """Round fp32 -> fp20 (1+8+11 float32r), RNE, kept as fp32 bit pattern."""
    u = np.ascontiguousarray(a, dtype=np.float32).view(np.uint32)
    low = u & np.uint32(0x00000FFF)
    base = u & np.uint32(0xFFFFF000)
    half = np.uint32(0x800)
    lsb = (u >> np.uint32(12)) & np.uint32(1)
    round_up = (low > half) | ((low == half) & (lsb == 1))
    return (base + np.where(round_up, np.uint32(0x1000), np.uint32(0))).view(
        np.float32
    )


def _pack_wt(W: np.ndarray) -> np.ndarray:
    # [64, 4096] -> [128, 32*64] with row p, col k*64+e = W[e, k*128+p]
    return np.ascontiguousarray(
        W.astype(np.float32, copy=False)
        .T.reshape(N_CHUNK, 128, N_EXPERTS)
        .transpose(1, 0, 2)
        .reshape(128, N_CHUNK * N_EXPERTS)
    )


def _make_in_maps(x: np.ndarray, W: np.ndarray) -> list:
    x = np.asarray(x, dtype=np.float32)
    wt = _pack_wt(W)
    wh = _round_f32r(wt)
    wl = _round_f32r(wt - wh)
    return [
        {
            "xt": np.ascontiguousarray(x[c * TPC : (c + 1) * TPC].T),
            "wh": wh,
            "wl": wl,
        }
        for c in range(N_CORES)
    ]


def kernel(x: np.ndarray, W: np.ndarray) -> np.ndarray:
    nc = _get_program()
    in_maps = _make_in_maps(x, W)
    res = run_bass_kernel_spmd(nc, in_maps, core_ids=list(range(N_CORES)))
    out = np.concatenate(
        [
            res.results[c]["idx"]
            .reshape(128, N_BLK, TOP_K)
            .transpose(1, 0, 2)
            .reshape(TPC, TOP_K)
            for c in range(N_CORES)
        ],
        axis=0,
    )
    return out.astype(np.int32)


# revision 11
# speedup vs baseline: 1.0405x; 1.0405x over previous
"""MoE router (linear gate -> softmax -> top-8 indices) on 8 Trainium2 cores.

Strategy (data-parallel over tokens, W replicated):
  - Each core gets 2048 tokens. x is pre-transposed on the host so each core
    receives x^T [4096, 2048] — the PE needs the contraction dim (d_model) on
    partitions and fp32 has no DMA-transpose path, so transposing on-chip
    would double PE work.
  - Softmax is strictly monotonic, so top-k of softmax(logits) == top-k of
    logits; the softmax is skipped entirely.
  - The gate matmul runs in float32r (fp20: 1+8+11) which streams at 1
    cycle/row vs fp32's 4, using an exactly-compensated split:
        x = x_hi + x_lo,  W = w_hi + w_lo   (each half fp20-representable)
        logits = w_hi·x_hi + w_hi·x_lo + w_lo·x_hi   (3 passes, fp32 PSUM)
    The dropped w_lo·x_lo term is O(2^-24) relative — fp32-level accuracy
    (validated on HW: max err 1.5e-7 vs fp32's 1.2e-7). W is split on the
    host; x is split on-chip (ACT rounds to f32r, DVE subtracts) to keep HBM
    traffic at 4 bytes/element.
  - Streaming: 32 x 1 MiB DMAs (one 128-row contraction chunk each, 358 GB/s
    measured), per-chunk ACT/DVE split, PE accumulates 4 [64, 512] PSUM
    logit tiles across all 32 chunks.
  - Top-8: PE-transpose the [64, 512] logit tiles to [128 tokens, 64
    experts] in PSUM, then DVE Max8 / MaxIndex produce the 8 largest values
    and indices per token (descending, ties -> lowest index, matching
    jax.lax.top_k). Indices are staged in SBUF and written with one DMA.
"""

import numpy as np

import concourse.bass as bass
import concourse.mybir as mybir
import concourse.tile as tile
from concourse import bacc
from concourse.bass_utils import run_bass_kernel_spmd
from concourse.masks import make_identity

N_CORES = 8
N_TOKENS = 16384
D_MODEL = 4096
N_EXPERTS = 64
TOP_K = 8

TPC = N_TOKENS // N_CORES      # tokens per core (2048)
GROUP = 512                    # tokens per matmul (max 4-byte moving dim)
N_GROUPS = TPC // GROUP        # 4
N_CHUNK = D_MODEL // 128       # 32 contraction chunks
N_BLK = TPC // 128             # 16 x 128-token output blocks

F32 = mybir.dt.float32
F32R = mybir.dt.float32r
U32 = mybir.dt.uint32

_CACHE: dict = {}


def _build_program():
    nc = bacc.Bacc(
        "TRN2", target_bir_lowering=False, debug=False, num_devices=N_CORES
    )
    xt_d = nc.dram_tensor("xt", [D_MODEL, TPC], F32, kind="ExternalInput")
    # W^T pre-packed+split on host: [128, 32*64], row p, col k*64+e = W_{hi/lo}[e, k*128+p]
    wh_d = nc.dram_tensor("wh", [128, N_CHUNK * N_EXPERTS], F32R, kind="ExternalInput")
    wl_d = nc.dram_tensor("wl", [128, N_CHUNK * N_EXPERTS], F32R, kind="ExternalInput")
    # idx laid out [128 partitions, 16 blocks, 8] — host unpermutes to [2048, 8]
    idx_d = nc.dram_tensor("idx", [128, N_BLK * TOP_K], U32, kind="ExternalOutput")

    with tile.TileContext(nc) as tc:
        with (
            tc.tile_pool(name="const", bufs=1) as const_pool,
            tc.tile_pool(name="xin", bufs=5) as x_pool,
            tc.tile_pool(name="hi", bufs=4) as hi_pool,
            tc.tile_pool(name="lo", bufs=4) as lo_pool,
            tc.tile_pool(name="lg_ps", bufs=1, space="PSUM") as lg_ps_pool,
            tc.tile_pool(name="lt_ps", bufs=4, space="PSUM") as lt_ps_pool,
            tc.tile_pool(name="small", bufs=2 * 4) as small_pool,
        ):
            ident = const_pool.tile([128, 128], F32)
            make_identity(nc, ident[:])
            # W DMAs go on the scalar HWDGE ring so x chunk 0 (sync ring)
            # isn't queued behind them.
            wh_sb = const_pool.tile([128, N_CHUNK, N_EXPERTS], F32R)
            nc.scalar.dma_start(
                wh_sb[:], wh_d.ap().rearrange("p (k e) -> p k e", k=N_CHUNK)
            )
            wl_sb = const_pool.tile([128, N_CHUNK, N_EXPERTS], F32R)
            nc.scalar.dma_start(
                wl_sb[:], wl_d.ap().rearrange("p (k e) -> p k e", k=N_CHUNK)
            )
            idx_stage = const_pool.tile([128, N_BLK, TOP_K], U32)

            lg_ps = [
                lg_ps_pool.tile([N_EXPERTS, GROUP], F32, name=f"lg{g}", tag=f"lg{g}")
                for g in range(N_GROUPS)
            ]

            xt_view = xt_d.ap().rearrange("(k p) t -> p k t", p=128)
            for k in range(N_CHUNK):
                x_sb = x_pool.tile([128, TPC], F32)
                nc.sync.dma_start(x_sb[:], xt_view[:, k, :])
                hi = hi_pool.tile([128, TPC], F32R)
                nc.scalar.copy(hi[:], x_sb[:])
                lo = lo_pool.tile([128, TPC], F32R)
                nc.vector.tensor_tensor(
                    lo[:], x_sb[:], hi[:].bitcast(F32), mybir.AluOpType.subtract
                )
                if k < N_CHUNK - 1:
                    for g in range(N_GROUPS):
                        sl = slice(g * GROUP, (g + 1) * GROUP)
                        nc.tensor.matmul(
                            lg_ps[g][:], wh_sb[:, k], hi[:, sl],
                            start=(k == 0), stop=False,
                        )
                    for g in range(N_GROUPS):
                        sl = slice(g * GROUP, (g + 1) * GROUP)
                        nc.tensor.matmul(
                            lg_ps[g][:], wh_sb[:, k], lo[:, sl],
                            start=False, stop=False,
                        )
                    for g in range(N_GROUPS):
                        sl = slice(g * GROUP, (g + 1) * GROUP)
                        nc.tensor.matmul(
                            lg_ps[g][:], wl_sb[:, k], hi[:, sl],
                            start=False, stop=False,
                        )
                else:
                    # last chunk: finish group-by-group so each group's
                    # top-k tail overlaps the remaining groups' matmuls
                    for g in range(N_GROUPS):
                        sl = slice(g * GROUP, (g + 1) * GROUP)
                        nc.tensor.matmul(
                            lg_ps[g][:], wh_sb[:, k], hi[:, sl],
                            start=False, stop=False,
                        )
                        nc.tensor.matmul(
                            lg_ps[g][:], wh_sb[:, k], lo[:, sl],
                            start=False, stop=False,
                        )
                        nc.tensor.matmul(
                            lg_ps[g][:], wl_sb[:, k], hi[:, sl],
                            start=False, stop=True,
                        )

            for g in range(N_GROUPS):
                lg_sb = small_pool.tile([N_EXPERTS, GROUP], F32, tag="lgsb")
                nc.scalar.copy(lg_sb[:], lg_ps[g][:])
                for b in range(GROUP // 128):
                    lt_ps = lt_ps_pool.tile([128, N_EXPERTS], F32)
                    nc.tensor.transpose(
                        lt_ps[:],
                        lg_sb[:, b * 128 : (b + 1) * 128],
                        ident[:N_EXPERTS, :N_EXPERTS],
                    )
                    vals = small_pool.tile([128, TOP_K], F32, tag="vals")
                    nc.vector.max(vals[:], lt_ps[:])
                    nc.vector.max_index(
                        idx_stage[:, g * (GROUP // 128) + b, :], vals[:], lt_ps[:]
                    )

            nc.sync.dma_start(
                idx_d.ap().rearrange("p (b k) -> p b k", b=N_BLK), idx_stage[:]
            )

    nc.compile()
    return nc


def _get_program():
    if "nc" not in _CACHE:
        _CACHE["nc"] = _build_program()
    return _CACHE["nc"]


def _round_f32r(a: np.ndarray) -> np.ndarray:
    """Round fp32 -> fp20 (1+8+11 float32r), RNE, kept as fp32 bit pattern."""
    u = np.ascontiguousarray(a, dtype=np.float32).view(np.uint32)
    low = u & np.uint32(0x00000FFF)
    base = u & np.uint32(0xFFFFF000)
    half = np.uint32(0x800)
    lsb = (u >> np.uint32(12)) & np.uint32(1)
    round_up = (low > half) | ((low == half) & (lsb == 1))
    return (base + np.where(round_up, np.uint32(0x1000), np.uint32(0))).view(
        np.float32
    )


def _pack_wt(W: np.ndarray) -> np.ndarray:
    # [64, 4096] -> [128, 32*64] with row p, col k*64+e = W[e, k*128+p]
    return np.ascontiguousarray(
        W.astype(np.float32, copy=False)
        .T.reshape(N_CHUNK, 128, N_EXPERTS)
        .transpose(1, 0, 2)
        .reshape(128, N_CHUNK * N_EXPERTS)
    )


def _make_in_maps(x: np.ndarray, W: np.ndarray) -> list:
    x = np.asarray(x, dtype=np.float32)
    wt = _pack_wt(W)
    wh = _round_f32r(wt)
    wl = _round_f32r(wt - wh)
    return [
        {
            "xt": np.ascontiguousarray(x[c * TPC : (c + 1) * TPC].T),
            "wh": wh,
            "wl": wl,
        }
        for c in range(N_CORES)
    ]


def kernel(x: np.ndarray, W: np.ndarray) -> np.ndarray:
    nc = _get_program()
    in_maps = _make_in_maps(x, W)
    res = run_bass_kernel_spmd(nc, in_maps, core_ids=list(range(N_CORES)))
    out = np.concatenate(
        [
            res.results[c]["idx"]
            .reshape(128, N_BLK, TOP_K)
            .transpose(1, 0, 2)
            .reshape(TPC, TOP_K)
            for c in range(N_CORES)
        ],
        axis=0,
    )
    return out.astype(np.int32)


# revision 13
# speedup vs baseline: 1.2282x; 1.1804x over previous
"""MoE router (linear gate -> softmax -> top-8 indices) on 8 Trainium2 cores.

Strategy (data-parallel over tokens, W replicated):
  - Each core gets 2048 tokens. x is pre-transposed on the host so each core
    receives x^T [4096, 2048] — the PE needs the contraction dim (d_model) on
    partitions and fp32 has no DMA-transpose path, so transposing on-chip
    would double PE work.
  - Softmax is strictly monotonic, so top-k of softmax(logits) == top-k of
    logits; the softmax is skipped entirely.
  - The gate matmul runs in float32r (fp20: 1+8+11) which streams at 1
    cycle/row vs fp32's 4, using an exactly-compensated split:
        x = x_hi + x_lo,  W = w_hi + w_lo   (each half fp20-representable)
        logits = w_hi·x_hi + w_hi·x_lo + w_lo·x_hi   (fp32 PSUM)
    The dropped w_lo·x_lo term is O(2^-24) relative — fp32-level accuracy
    (validated on HW: max err 1.5e-7 vs fp32's 1.2e-7). W is split on the
    host; x is split on-chip (ACT rounds to f32r, DVE subtracts) to keep HBM
    traffic at 4 bytes/element.
  - PE work is 2 passes per chunk, not 3: the stationary is [w_hi | w_lo]
    [128, 128], so pass A (moving x_hi) yields w_hi·x_hi in PSUM rows 0-63
    AND w_lo·x_hi in rows 64-127 from one moving stream; pass B (moving
    x_lo) uses only the w_hi half into rows 0-63. The two halves are summed
    after the tail transpose, where they sit in the free dim.
  - Streaming: 32 x 1 MiB DMAs (one 128-row contraction chunk each, 358 GB/s
    measured), per-chunk ACT/DVE split, PE accumulates 4 [128, 512] PSUM
    logit tiles across all 32 chunks.
  - Top-8: PE-transpose the logit tiles to [128 tokens, 128], DVE-add the
    two 64-wide halves, then DVE Max8 / MaxIndex produce the 8 largest
    values and indices per token (descending, ties -> lowest index, matching
    jax.lax.top_k). Indices are staged in SBUF and written with one DMA.
"""

import numpy as np

import concourse.bass as bass
import concourse.mybir as mybir
import concourse.tile as tile
from concourse import bacc
from concourse.bass_utils import run_bass_kernel_spmd
from concourse.masks import make_identity

N_CORES = 8
N_TOKENS = 16384
D_MODEL = 4096
N_EXPERTS = 64
TOP_K = 8

TPC = N_TOKENS // N_CORES      # tokens per core (2048)
GROUP = 512                    # tokens per matmul (max 4-byte moving dim)
N_GROUPS = TPC // GROUP        # 4
N_CHUNK = D_MODEL // 128       # 32 contraction chunks
N_BLK = TPC // 128             # 16 x 128-token output blocks

F32 = mybir.dt.float32
F32R = mybir.dt.float32r
U32 = mybir.dt.uint32

_CACHE: dict = {}


def _build_program():
    nc = bacc.Bacc(
        "TRN2", target_bir_lowering=False, debug=False, num_devices=N_CORES
    )
    xt_d = nc.dram_tensor("xt", [D_MODEL, TPC], F32, kind="ExternalInput")
    # [w_hi | w_lo] packed on host: [128, 32, 128] with
    # [p, k, e]      = W_hi[e, k*128+p]  for e < 64
    # [p, k, 64+e]   = W_lo[e, k*128+p]
    ww_d = nc.dram_tensor(
        "ww", [128, N_CHUNK * 2 * N_EXPERTS], F32R, kind="ExternalInput"
    )
    # idx laid out [128 partitions, 16 blocks, 8] — host unpermutes to [2048, 8]
    idx_d = nc.dram_tensor("idx", [128, N_BLK * TOP_K], U32, kind="ExternalOutput")

    with tile.TileContext(nc) as tc:
        with (
            tc.tile_pool(name="const", bufs=1) as const_pool,
            tc.tile_pool(name="xin", bufs=5) as x_pool,
            tc.tile_pool(name="hi", bufs=4) as hi_pool,
            tc.tile_pool(name="lo", bufs=4) as lo_pool,
            tc.tile_pool(name="lg_ps", bufs=1, space="PSUM") as lg_ps_pool,
            tc.tile_pool(name="lt_ps", bufs=2, space="PSUM") as lt_ps_pool,
            tc.tile_pool(name="small", bufs=2 * 4) as small_pool,
        ):
            ident = const_pool.tile([128, 128], F32)
            make_identity(nc, ident[:])
            # W DMA goes on the scalar HWDGE ring so x chunk 0 (sync ring)
            # isn't queued behind it.
            ww_sb = const_pool.tile([128, N_CHUNK, 2 * N_EXPERTS], F32R)
            nc.scalar.dma_start(
                ww_sb[:], ww_d.ap().rearrange("p (k e) -> p k e", k=N_CHUNK)
            )
            idx_stage = const_pool.tile([128, N_BLK, TOP_K], U32)

            lg_ps = [
                lg_ps_pool.tile(
                    [2 * N_EXPERTS, GROUP], F32, name=f"lg{g}", tag=f"lg{g}"
                )
                for g in range(N_GROUPS)
            ]

            xt_view = xt_d.ap().rearrange("(k p) t -> p k t", p=128)
            for k in range(N_CHUNK):
                x_sb = x_pool.tile([128, TPC], F32)
                nc.sync.dma_start(x_sb[:], xt_view[:, k, :])
                hi = hi_pool.tile([128, TPC], F32R)
                nc.scalar.copy(hi[:], x_sb[:])
                lo = lo_pool.tile([128, TPC], F32R)
                nc.vector.tensor_tensor(
                    lo[:], x_sb[:], hi[:].bitcast(F32), mybir.AluOpType.subtract
                )
                # pass A: [w_hi|w_lo]·hi -> all 128 PSUM rows
                # pass B: w_hi·lo -> rows 0-63 only
                # chunk 0 must OPEN each tile with a full-tile start;
                # chunk 31 must CLOSE each tile with a full-tile stop.
                for g in range(N_GROUPS):
                    sl = slice(g * GROUP, (g + 1) * GROUP)
                    if k == 0:
                        nc.tensor.matmul(
                            lg_ps[g][:], ww_sb[:, k], hi[:, sl],
                            start=True, stop=False,
                        )
                        nc.tensor.matmul(
                            lg_ps[g][: N_EXPERTS], ww_sb[:, k, :N_EXPERTS],
                            lo[:, sl], start=False, stop=False,
                        )
                    elif k < N_CHUNK - 1:
                        nc.tensor.matmul(
                            lg_ps[g][:], ww_sb[:, k], hi[:, sl],
                            start=False, stop=False,
                        )
                        nc.tensor.matmul(
                            lg_ps[g][: N_EXPERTS], ww_sb[:, k, :N_EXPERTS],
                            lo[:, sl], start=False, stop=False,
                        )
                    else:
                        nc.tensor.matmul(
                            lg_ps[g][: N_EXPERTS], ww_sb[:, k, :N_EXPERTS],
                            lo[:, sl], start=False, stop=False,
                        )
                        nc.tensor.matmul(
                            lg_ps[g][:], ww_sb[:, k], hi[:, sl],
                            start=False, stop=True,
                        )

            for g in range(N_GROUPS):
                lg_sb = small_pool.tile([2 * N_EXPERTS, GROUP], F32, tag="lgsb")
                nc.scalar.copy(lg_sb[:], lg_ps[g][:])
                for b in range(GROUP // 128):
                    lt_ps = lt_ps_pool.tile([128, 2 * N_EXPERTS], F32)
                    nc.tensor.transpose(
                        lt_ps[:],
                        lg_sb[:, b * 128 : (b + 1) * 128],
                        ident[:],
                    )
                    lt_h = small_pool.tile([128, N_EXPERTS], F32, tag="lth")
                    nc.scalar.copy(lt_h[:], lt_ps[:, :N_EXPERTS])
                    lt_sb = small_pool.tile([128, N_EXPERTS], F32, tag="ltsb")
                    nc.vector.tensor_tensor(
                        lt_sb[:],
                        lt_h[:],
                        lt_ps[:, N_EXPERTS:],
                        mybir.AluOpType.add,
                    )
                    vals = small_pool.tile([128, TOP_K], F32, tag="vals")
                    nc.vector.max(vals[:], lt_sb[:])
                    nc.vector.max_index(
                        idx_stage[:, g * (GROUP // 128) + b, :], vals[:], lt_sb[:]
                    )

            nc.sync.dma_start(
                idx_d.ap().rearrange("p (b k) -> p b k", b=N_BLK), idx_stage[:]
            )

    nc.compile()
    return nc


def _get_program():
    if "nc" not in _CACHE:
        _CACHE["nc"] = _build_program()
    return _CACHE["nc"]


def _round_f32r(a: np.ndarray) -> np.ndarray:
    """Round fp32 -> fp20 (1+8+11 float32r), RNE, kept as fp32 bit pattern."""
    u = np.ascontiguousarray(a, dtype=np.float32).view(np.uint32)
    low = u & np.uint32(0x00000FFF)
    base = u & np.uint32(0xFFFFF000)
    half = np.uint32(0x800)
    lsb = (u >> np.uint32(12)) & np.uint32(1)
    round_up = (low > half) | ((low == half) & (lsb == 1))
    return (base + np.where(round_up, np.uint32(0x1000), np.uint32(0))).view(
        np.float32
    )


def _pack_ww(W: np.ndarray) -> np.ndarray:
    # [64, 4096] -> [128, 32*128]: [p, k*128+e] = W_hi[e, k*128+p],
    #                              [p, k*128+64+e] = W_lo[e, k*128+p]
    wt = (
        W.astype(np.float32, copy=False)
        .T.reshape(N_CHUNK, 128, N_EXPERTS)
        .transpose(1, 0, 2)
    )  # [128, 32, 64]
    wh = _round_f32r(wt)
    wl = _round_f32r(wt - wh)
    ww = np.concatenate([wh.reshape(128, N_CHUNK, N_EXPERTS),
                         wl.reshape(128, N_CHUNK, N_EXPERTS)], axis=2)
    return np.ascontiguousarray(ww.reshape(128, N_CHUNK * 2 * N_EXPERTS))


def _make_in_maps(x: np.ndarray, W: np.ndarray) -> list:
    x = np.asarray(x, dtype=np.float32)
    ww = _pack_ww(W)
    return [
        {
            "xt": np.ascontiguousarray(x[c * TPC : (c + 1) * TPC].T),
            "ww": ww,
        }
        for c in range(N_CORES)
    ]


def kernel(x: np.ndarray, W: np.ndarray) -> np.ndarray:
    nc = _get_program()
    in_maps = _make_in_maps(x, W)
    res = run_bass_kernel_spmd(nc, in_maps, core_ids=list(range(N_CORES)))
    out = np.concatenate(
        [
            res.results[c]["idx"]
            .reshape(128, N_BLK, TOP_K)
            .transpose(1, 0, 2)
            .reshape(TPC, TOP_K)
            for c in range(N_CORES)
        ],
        axis=0,
    )
    return out.astype(np.int32)
